# revision 1
# baseline (speedup 1.0000x reference)
"""Trainium2 Bass kernel for the NNConv/GNN message-passing problem.

Strategy (graph-parallel over 8 cores, 128 graphs each):
  * Edge features take only 8^3=512 distinct values -> the edge-conditioned
    weight MLP (99% of reference FLOPs) is deduplicated into a 512-entry
    table of [64,32] matrices, built on-device with small GEMMs.
  * Node encoder: one dma_gather over a stacked [1152,128]-padded bf16
    embedding table + 8 vector adds.
  * Messages: edges type-sorted (host-computed permutation); per-type
    matmul  msg[n_t,32] = XS_T[64,n_t].T @ Wtab[t].
  * segment_sum + root + bias: per-graph one-hot matmul
    aggT[32,40] = msg_g[128,32].T @ Dhat_g[128,40], accumulated into a
    transposed feature plane F[32,5120] initialized with x@root + bias.
  * Readout MLP runs transposed (features on partitions, graphs on free),
    biases applied per-partition by the scalar engine.
"""

import numpy as np
import ml_dtypes

import concourse.bass as bass
import concourse.bacc as bacc
import concourse.mybir as mybir
import concourse.tile as tile
from concourse import library_config
from concourse.bass_utils import run_bass_kernel_spmd

BF16 = ml_dtypes.bfloat16
F32 = np.float32

G, NPG, EPG, MAXN = 1024, 40, 80, 51
D_IN, D_OUT, D_EDGE = 64, 32, 16
NCORES = 8
GPC = G // NCORES          # 128 graphs / core
NPC = GPC * NPG            # 5120 nodes / core
EPC = GPC * EPG            # 10240 edges / core
NTYPES = 512
VOC = 9 * 128              # stacked atom-embedding rows


def _wrap_idx(idx):
    """int16 index array -> [128, n/16] layout for dma_gather (16-partition
    wrap, replicated for the 8 gpsimd cores)."""
    idx = np.asarray(idx, np.int16)
    n = idx.shape[0]
    assert n % 16 == 0
    w = np.empty((128, n // 16), np.int16)
    for p in range(16):
        w[p::16, :] = idx[p::16]
    return w


def _build_program(C):
    """Emit the SPMD Tile program. C = per-type capacity (multiple of 64)."""
    dt = mybir.dt
    nc = bacc.Bacc("TRN2", target_bir_lowering=False, debug=False)

    NXS = NTYPES * C           # type-padded edge columns
    NXT = NXS + NPC            # + identity (x.T) columns
    TPC = C // 64              # 64-col type-slices per type is C/64... (C=64 -> 1)
    assert C % 64 == 0
    CHUNKS = NXS // 128        # msg psum chunks of 128 rows

    # ---- DRAM I/O ----
    atab = nc.dram_tensor("atab", [VOC, 128], dt.bfloat16, kind="ExternalInput")
    enc_idx = nc.dram_tensor("enc_idx", [128, 9 * NPC // 16], dt.int16, kind="ExternalInput")
    xt_idx = nc.dram_tensor("xt_idx", [128, NXT // 16], dt.int16, kind="ExternalInput")
    rg_idx = nc.dram_tensor("rg_idx", [128, GPC * 128 // 16], dt.int16, kind="ExternalInput")
    w1eff = nc.dram_tensor("w1eff", [24, 1024], dt.bfloat16, kind="ExternalInput")
    oh24 = nc.dram_tensor("oh24", [24, 512], dt.bfloat16, kind="ExternalInput")
    gw2 = nc.dram_tensor("gw2", [128, 8, 256], dt.bfloat16, kind="ExternalInput")
    gw3p = nc.dram_tensor("gw3p", [128, 2, 32, 64], dt.bfloat16, kind="ExternalInput")
    rootp = nc.dram_tensor("rootp", [128, 32], dt.bfloat16, kind="ExternalInput")
    cbias = nc.dram_tensor("cbias", [32, 1], dt.float32, kind="ExternalInput")
    dhat = nc.dram_tensor("dhat", [128, GPC, 40], dt.float32, kind="ExternalInput")
    w1 = nc.dram_tensor("w1", [128, 10, 256], dt.bfloat16, kind="ExternalInput")
    w2 = nc.dram_tensor("w2", [128, 2, 128], dt.bfloat16, kind="ExternalInput")
    w3 = nc.dram_tensor("w3", [128, 32], dt.bfloat16, kind="ExternalInput")
    w4 = nc.dram_tensor("w4", [32, 8], dt.bfloat16, kind="ExternalInput")
    w5 = nc.dram_tensor("w5", [8, 1], dt.bfloat16, kind="ExternalInput")
    mb1 = nc.dram_tensor("mb1", [128, 2], dt.float32, kind="ExternalInput")
    mb2 = nc.dram_tensor("mb2", [128, 1], dt.float32, kind="ExternalInput")
    mb3 = nc.dram_tensor("mb3", [32, 1], dt.float32, kind="ExternalInput")
    mb4 = nc.dram_tensor("mb4", [8, 1], dt.float32, kind="ExternalInput")
    mb5 = nc.dram_tensor("mb5", [1, 1], dt.float32, kind="ExternalInput")

    x_dram = nc.dram_tensor("x_scr", [NPC + 128, 128], dt.bfloat16)
    msg_dram = nc.dram_tensor("msg_scr", [NXS, 64], dt.float32)
    y = nc.dram_tensor("y", [1, GPC], dt.float32, kind="ExternalOutput")

    with tile.TileContext(nc) as tc:
        ch_reg = [None]

        def chunked_gather(dst3, srcT, idx, total, elem, transpose=False):
            CH = 512
            assert total % CH == 0
            if ch_reg[0] is None:
                ch_reg[0] = nc.gpsimd.to_reg(CH)
            for k in range(total // CH):
                isl = idx[:, k * (CH // 16):(k + 1) * (CH // 16)]
                if transpose:
                    osl = dst3[:, :, k * CH:(k + 1) * CH]
                else:
                    osl = dst3[:, k * (CH // 128):(k + 1) * (CH // 128), :]
                nc.gpsimd.dma_gather(osl, srcT, isl, CH, ch_reg[0], elem,
                                     transpose=transpose)

        nc.gpsimd.load_library(library_config.mlp)

        with tc.tile_pool(name="persist", bufs=1) as pp:
            # ---- persistent weight tiles ----
            w1eff_sb = pp.tile([24, 1024], dt.bfloat16)
            nc.sync.dma_start(w1eff_sb[:], w1eff[:])
            oh24_sb = pp.tile([24, 512], dt.bfloat16)
            nc.sync.dma_start(oh24_sb[:], oh24[:])
            gw2_sb = pp.tile([128, 8, 256], dt.bfloat16)
            nc.sync.dma_start(gw2_sb[:], gw2[:])
            gw3p_sb = pp.tile([128, 2, 32, 64], dt.bfloat16)
            nc.sync.dma_start(gw3p_sb[:], gw3p[:])
            rootp_sb = pp.tile([128, 32], dt.bfloat16)
            nc.sync.dma_start(rootp_sb[:], rootp[:])
            cbias_sb = pp.tile([32, 1], dt.float32)
            nc.sync.dma_start(cbias_sb[:], cbias[:])
            wtab = pp.tile([64, NTYPES, 32], dt.bfloat16)
            F = pp.tile([32, NPC], dt.float32)

            # ---- Wtable: h1T -> h2T -> per-o slices ----
            tp_cm = tc.tile_pool(name="tabp", bufs=1)
            tp = tp_cm.__enter__()
            psp_cm = tc.tile_pool(name="ps_tab", bufs=3, space="PSUM")
            psp = psp_cm.__enter__()
            h1t = tp.tile([128, 8, 512], dt.bfloat16)
            for k8 in range(8):
                ps = psp.tile([128, 512], dt.float32, tag="tab")
                nc.tensor.matmul(ps[:], w1eff_sb[:, k8 * 128:(k8 + 1) * 128],
                                 oh24_sb[:], start=True, stop=True)
                nc.scalar.activation(h1t[:, k8, :], ps[:],
                                     mybir.ActivationFunctionType.Relu)
            h2t = tp.tile([128, 2, 512], dt.bfloat16)
            for m2 in range(2):
                ps = psp.tile([128, 512], dt.float32, tag="tab")
                for k8 in range(8):
                    nc.tensor.matmul(ps[:], gw2_sb[:, k8, m2 * 128:(m2 + 1) * 128],
                                     h1t[:, k8, :], start=(k8 == 0), stop=(k8 == 7))
                nc.scalar.activation(h2t[:, m2, :], ps[:],
                                     mybir.ActivationFunctionType.Relu)
            for o in range(32):
                ps = psp.tile([64, 512], dt.float32, tag="tab2")
                for k2 in range(2):
                    nc.tensor.matmul(ps[:], gw3p_sb[:, k2, o, :], h2t[:, k2, :],
                                     start=(k2 == 0), stop=(k2 == 1))
                nc.vector.tensor_copy(wtab[:, :, o], ps[:])
            psp_cm.__exit__(None, None, None)
            tp_cm.__exit__(None, None, None)

            # ---- encoder: gather 9 embedding rows/node in 3 passes, sum ----
            with tc.tile_pool(name="enc", bufs=1) as ep:
                eidx = ep.tile([128, 9 * NPC // 16], dt.int16)
                nc.sync.dma_start(eidx[:], enc_idx[:])
                S = NPC // 128  # 40 slots per feature column
                x_bf = ep.tile([128, S, 128], dt.bfloat16)
                NB = 3 * NPC
                for b in range(3):
                    epart = ep.tile([128, NB // 128, 128], dt.bfloat16,
                                    tag="epart")
                    chunked_gather(
                        epart[:], atab[:],
                        eidx[:, b * (NB // 16):(b + 1) * (NB // 16)],
                        NB, 128)
                    if b == 0:
                        nc.vector.tensor_tensor(
                            x_bf[:], epart[:, 0:S, :], epart[:, S:2 * S, :],
                            op=mybir.AluOpType.add)
                        nc.vector.tensor_tensor(
                            x_bf[:], x_bf[:], epart[:, 2 * S:3 * S, :],
                            op=mybir.AluOpType.add)
                    else:
                        for j in range(3):
                            nc.vector.tensor_tensor(
                                x_bf[:], x_bf[:], epart[:, j * S:(j + 1) * S, :],
                                op=mybir.AluOpType.add)
                # stage x rows (+ one zero block) to DRAM for the src-gather
                xv = x_dram.ap().rearrange("(s p) d -> p s d", p=128)
                nc.sync.dma_start(xv[:, 0:S, :], x_bf[:])
                zrow = ep.tile([128, 1, 128], dt.bfloat16)
                nc.vector.memset(zrow[:], 0.0)
                nc.sync.dma_start(xv[:, S:S + 1, :], zrow[:])

            # ---- transposed gather: XS_T (type-sorted) ++ x.T ----
            xtp_cm = tc.tile_pool(name="xtp", bufs=1)
            xp = xtp_cm.__enter__()
            xt = xp.tile([128, 1, NXT], dt.bfloat16)
            xidx = xp.tile([128, NXT // 16], dt.int16)
            nc.sync.dma_start(xidx[:], xt_idx[:])
            chunked_gather(xt[:], x_dram[:], xidx[:], NXT, 128, transpose=True)
            xtv = xt[:, 0, :]

            # ---- F init: x @ root + conv_bias (transposed) ----
            psp_cm = tc.tile_pool(name="ps_mid", bufs=3, space="PSUM")
            psp = psp_cm.__enter__()
            for nch in range(NPC // 512):
                ps = psp.tile([32, 512], dt.float32, tag="xr")
                nc.tensor.matmul(ps[:], rootp_sb[:],
                                 xtv[:, NXS + nch * 512: NXS + (nch + 1) * 512],
                                 start=True, stop=True)
                nc.scalar.activation(F[:, nch * 512:(nch + 1) * 512], ps[:],
                                     mybir.ActivationFunctionType.Identity,
                                     bias=cbias_sb[:])

            # ---- messages: per-type matmuls, staged to DRAM ----
            with tc.tile_pool(name="msgp", bufs=6) as mp:
                msgv = msg_dram.ap().rearrange("(s p) d -> p s d", p=128)
                for ch in range(CHUNKS):
                    ps = psp.tile([128, 32], dt.float32, tag="msg")
                    for half in range(128 // 64):
                        col = ch * 128 + half * 64
                        nc.tensor.matmul(ps[half * 64:(half + 1) * 64, :],
                                         xtv[0:64, col:col + 64],
                                         wtab[:, col // C, :],
                                         start=True, stop=True)
                    st = mp.tile([128, 32], dt.float32, tag="stage")
                    nc.vector.tensor_copy(st[:], ps[:])
                    nc.sync.dma_start(msgv[:, ch, 0:32], st[:])
            psp_cm.__exit__(None, None, None)
            xtp_cm.__exit__(None, None, None)

            # ---- regather per graph (128 rows each) + scatter matmul ----
            with tc.tile_pool(name="scat", bufs=1) as sp:
                ridx = sp.tile([128, GPC * 128 // 16], dt.int16)
                nc.sync.dma_start(ridx[:], rg_idx[:])
                gt = sp.tile([128, GPC, 64], dt.float32)
                chunked_gather(gt[:], msg_dram[:], ridx[:], GPC * 128, 64)
                dhat_sb = sp.tile([128, GPC, 40], dt.float32)
                nc.sync.dma_start(dhat_sb[:], dhat[:])
                psp_cm = tc.tile_pool(name="ps_sc", bufs=6, space="PSUM")
                psp = psp_cm.__enter__()
                for g in range(GPC):
                    ps = psp.tile([32, 40], dt.float32, tag="sc")
                    nc.tensor.matmul(ps[:], gt[:, g, 0:32], dhat_sb[:, g, :],
                                     start=True, stop=True)
                    nc.vector.tensor_tensor(F[:, g * 40:(g + 1) * 40],
                                            F[:, g * 40:(g + 1) * 40], ps[:],
                                            op=mybir.AluOpType.add)
                psp_cm.__exit__(None, None, None)

            # ---- fold F[32,5120] -> F2[128,1280] (bf16) ----
            with tc.tile_pool(name="ro", bufs=1) as rp:
                F2 = rp.tile([128, GPC * 10], dt.bfloat16)
                Fv = F[:].rearrange("p (g q j) -> p g q j", g=GPC, q=10)
                for j in range(4):
                    dst = F2[j * 32:(j + 1) * 32, :].rearrange(
                        "p (g q) -> p g q", g=GPC)
                    nc.vector.tensor_copy(dst, Fv[:, :, :, j])

                # ---- readout MLP (transposed, biases per-partition) ----
                w1_sb = rp.tile([128, 10, 256], dt.bfloat16)
                nc.sync.dma_start(w1_sb[:], w1[:])
                w2_sb = rp.tile([128, 2, 128], dt.bfloat16)
                nc.sync.dma_start(w2_sb[:], w2[:])
                w3_sb = rp.tile([128, 32], dt.bfloat16)
                nc.sync.dma_start(w3_sb[:], w3[:])
                w4_sb = rp.tile([32, 8], dt.bfloat16)
                nc.sync.dma_start(w4_sb[:], w4[:])
                w5_sb = rp.tile([8, 1], dt.bfloat16)
                nc.sync.dma_start(w5_sb[:], w5[:])
                mb1_sb = rp.tile([128, 2], dt.float32)
                nc.sync.dma_start(mb1_sb[:], mb1[:])
                mb2_sb = rp.tile([128, 1], dt.float32)
                nc.sync.dma_start(mb2_sb[:], mb2[:])
                mb3_sb = rp.tile([32, 1], dt.float32)
                nc.sync.dma_start(mb3_sb[:], mb3[:])
                mb4_sb = rp.tile([8, 1], dt.float32)
                nc.sync.dma_start(mb4_sb[:], mb4[:])
                mb5_sb = rp.tile([1, 1], dt.float32)
                nc.sync.dma_start(mb5_sb[:], mb5[:])

                psp_cm = tc.tile_pool(name="ps_ro", bufs=2, space="PSUM")
                psp = psp_cm.__enter__()
                F2q = F2[:].rearrange("p (g q) -> p q g", q=10)
                a1 = rp.tile([128, 2, GPC], dt.bfloat16)
                for mh in range(2):
                    ps = psp.tile([128, GPC], dt.float32, tag="ro1")
                    for q in range(10):
                        nc.tensor.matmul(ps[:], w1_sb[:, q, mh * 128:(mh + 1) * 128],
                                         F2q[:, q, :], start=(q == 0), stop=(q == 9))
                    nc.scalar.activation(a1[:, mh, :], ps[:],
                                         mybir.ActivationFunctionType.Relu,
                                         bias=mb1_sb[:, mh:mh + 1])
                ps2 = psp.tile([128, GPC], dt.float32, tag="ro1")
                for h in range(2):
                    nc.tensor.matmul(ps2[:], w2_sb[:, h, :], a1[:, h, :],
                                     start=(h == 0), stop=(h == 1))
                a2 = rp.tile([128, GPC], dt.bfloat16)
                nc.scalar.activation(a2[:], ps2[:],
                                     mybir.ActivationFunctionType.Relu,
                                     bias=mb2_sb[:])
                ps3 = psp.tile([32, GPC], dt.float32, tag="ro2")
                nc.tensor.matmul(ps3[:], w3_sb[:], a2[:], start=True, stop=True)
                a3 = rp.tile([32, GPC], dt.bfloat16)
                nc.scalar.activation(a3[:], ps3[:],
                                     mybir.ActivationFunctionType.Relu,
                                     bias=mb3_sb[:])
                ps4 = psp.tile([8, GPC], dt.float32, tag="ro2")
                nc.tensor.matmul(ps4[:], w4_sb[:], a3[:], start=True, stop=True)
                a4 = rp.tile([8, GPC], dt.bfloat16)
                nc.scalar.activation(a4[:], ps4[:],
                                     mybir.ActivationFunctionType.Relu,
                                     bias=mb4_sb[:])
                ps5 = psp.tile([1, GPC], dt.float32, tag="ro2")
                nc.tensor.matmul(ps5[:], w5_sb[:], a4[:], start=True, stop=True)
                yv = rp.tile([1, GPC], dt.float32)
                nc.scalar.activation(yv[:], ps5[:],
                                     mybir.ActivationFunctionType.Identity,
                                     bias=mb5_sb[:])
                nc.sync.dma_start(y[:], yv[:])
                psp_cm.__exit__(None, None, None)

    nc.compile()
    return nc


def _host_prep(node_features, edge_features, edge_index, batch,
               atom_emb, bond_emb, gW1, gW2, gW3, root, conv_bias, mws, mbs):
    """Build per-core input maps + pick type capacity C."""
    nf = np.asarray(node_features, np.int64)
    ef = np.asarray(edge_features, np.int64)
    src = np.asarray(edge_index, np.int64)[0]
    dst = np.asarray(edge_index, np.int64)[1]
    atom_emb = np.asarray(atom_emb, F32)
    bond_emb = np.asarray(bond_emb, F32)
    gW1 = np.asarray(gW1, F32); gW2 = np.asarray(gW2, F32); gW3 = np.asarray(gW3, F32)
    root = np.asarray(root, F32); conv_bias = np.asarray(conv_bias, F32)
    mws = [np.asarray(w, F32) for w in mws]
    mbs = [np.asarray(b, F32) for b in mbs]

    # ---- replicated weight tensors ----
    atab = np.zeros((VOC, 128), BF16)
    atab[:, :64] = atom_emb.reshape(VOC, 64).astype(BF16)
    bemb_flat = bond_emb.reshape(24, D_EDGE)                       # [24,16]
    w1eff = (bemb_flat @ gW1).astype(BF16)                         # [24,1024]
    tt = np.arange(NTYPES)
    i0, i1, i2 = tt // 64, (tt // 8) % 8, tt % 8
    oh24 = np.zeros((24, NTYPES), BF16)
    oh24[i0, tt] = 1; oh24[8 + i1, tt] = 1; oh24[16 + i2, tt] = 1
    gw2r = gW2.reshape(8, 128, 256).transpose(1, 0, 2).astype(BF16)      # [128,8,256]
    gw3p = gW3.reshape(2, 128, 64, 32).transpose(1, 0, 3, 2).astype(BF16)  # [128,2,32,64] = [cp,k2,o,d]
    rootp = np.zeros((128, 32), BF16)
    rootp[:64] = root.astype(BF16)
    cbias = conv_bias.reshape(32, 1).astype(F32)
    # readout weights: w1 reordered [(j*32+oo), q, r] = mW1[(4q+j)*32+oo, r]
    w1r = mws[0][:1280].reshape(40, 32, 256).reshape(10, 4, 32, 256) \
        .transpose(1, 2, 0, 3).reshape(128, 10, 256).astype(BF16)
    w2r = mws[1].reshape(2, 128, 128).transpose(1, 0, 2).astype(BF16)
    w3r = mws[2].astype(BF16)                                      # [128,32]
    w4r = mws[3].astype(BF16)                                      # [32,8]
    w5r = mws[4].astype(BF16)                                      # [8,1]
    mb1r = mbs[0].reshape(2, 128).T.astype(F32)
    mb2r = mbs[1].reshape(128, 1).astype(F32)
    mb3r = mbs[2].reshape(32, 1).astype(F32)
    mb4r = mbs[3].reshape(8, 1).astype(F32)
    mb5r = mbs[4].reshape(1, 1).astype(F32)

    # ---- per-core data ----
    types = (ef[:, 0] * 64 + ef[:, 1] * 8 + ef[:, 2]).astype(np.int64)
    counts_all = np.zeros((NCORES, NTYPES), np.int64)
    for c in range(NCORES):
        counts_all[c] = np.bincount(types[c * EPC:(c + 1) * EPC], minlength=NTYPES)
    C = max(64, int(np.ceil(counts_all.max() / 64)) * 64)
    assert counts_all.min(axis=1).max() < C  # every core has a padded slot

    in_maps = []
    for c in range(NCORES):
        nsl = slice(c * NPC, (c + 1) * NPC)
        esl = slice(c * EPC, (c + 1) * EPC)
        nf_c = nf[nsl]
        t_c = types[esl]
        src_c = src[esl] - c * NPC
        dst_c = dst[esl] - c * NPC
        cnt = counts_all[c]

        # encoder gather indices, feature-column major
        eidx = (np.arange(9)[:, None] * 128 + nf_c.T).reshape(-1)   # [9*5120]

        # type-sort: edge e -> column t*C + rank
        order = np.argsort(t_c, kind="stable")
        rank = np.empty(EPC, np.int64)
        off = np.concatenate([[0], np.cumsum(cnt)[:-1]])
        rank[order] = np.arange(EPC) - off[t_c[order]]
        pos = t_c * C + rank                                        # [EPC]
        xs_idx = np.full(NTYPES * C, NPC, np.int64)                 # pad -> zero row
        xs_idx[pos] = src_c
        xt_i = np.concatenate([xs_idx, np.arange(NPC)])

        # regather: graph-order 128-row tiles (80 real + 48 pad)
        tmin = int(np.argmin(cnt))
        zslot = tmin * C + int(cnt[tmin])
        rg = np.full((GPC, 128), zslot, np.int64)
        rg[:, :80] = pos.reshape(GPC, 80)
        rg_i = rg.reshape(-1)

        # scatter one-hot [k, g, m]
        dh = np.zeros((128, GPC, 40), F32)
        kk = np.tile(np.arange(80), GPC)
        gg = np.repeat(np.arange(GPC), 80)
        dh[kk, gg, (dst_c - gg * NPG)] = 1.0

        in_maps.append(dict(
            atab=atab, enc_idx=_wrap_idx(eidx), xt_idx=_wrap_idx(xt_i),
            rg_idx=_wrap_idx(rg_i), w1eff=w1eff, oh24=oh24, gw2=gw2r,
            gw3p=gw3p, rootp=rootp, cbias=cbias, dhat=dh, w1=w1r, w2=w2r,
            w3=w3r, w4=w4r, w5=w5r, mb1=mb1r, mb2=mb2r, mb3=mb3r,
            mb4=mb4r, mb5=mb5r,
        ))
    return in_maps, C


def kernel(node_features, edge_features, edge_index, batch,
           atom_emb, bond_emb, gW1, gW2, gW3, root, conv_bias,
           mW1, mb1, mW2, mb2, mW3, mb3, mW4, mb4, mW5, mb5):
    in_maps, C = _host_prep(
        node_features, edge_features, edge_index, batch, atom_emb, bond_emb,
        gW1, gW2, gW3, root, conv_bias,
        [mW1, mW2, mW3, mW4, mW5], [mb1, mb2, mb3, mb4, mb5])
    nc = _build_program(C)
    res = run_bass_kernel_spmd(nc, in_maps, list(range(NCORES)))
    y = np.concatenate([r["y"].reshape(GPC) for r in res.results])
    return y.reshape(G, 1).astype(F32)



# revision 2
# speedup vs baseline: 2.3113x; 2.3113x over previous
"""Trainium2 Bass kernel for the NNConv/GNN message-passing problem.

Strategy (graph-parallel over 8 cores, 128 graphs each):
  * Edge features take only 8^3=512 distinct values -> the edge-conditioned
    weight MLP (99% of reference FLOPs) is deduplicated into a 512-entry
    table of [64,32] matrices, computed host-side (parameter-only
    transform) and shipped as part of one bf16 weight blob.
  * The replicated weight blob (atom table + W-table + readout weights,
    ~3.1 MB) is uploaded SHARDED 1/8th per core and AllGathered on-device
    over NeuronLink, cutting per-call host->device transfer 8x.
  * Gather index tensors are uploaded unreplicated [16, n/16] and
    replicated to the 8 gpsimd cores' partition groups on device.
  * The segment-sum one-hot (dhat) is built on device from compact int16
    dst indices with 40 is_equal compares.
  * Node encoder: dma_gather over the stacked [1152,128]-padded bf16
    embedding table + vector adds.
  * Messages: edges type-sorted (host-computed permutation); per-type
    matmul  msg[n_t,32] = XS_T[64,n_t].T @ Wtab[t].
  * segment_sum + root + bias: per-graph one-hot matmul
    aggT[32,40] = msg_g[128,32].T @ Dhat_g[128,40], accumulated into a
    transposed feature plane F[32,5120] initialized with x@root + bias.
  * Readout MLP runs transposed (features on partitions, graphs on free),
    biases applied per-partition by the scalar engine.
"""

import numpy as np
import ml_dtypes

import concourse.bass as bass
import concourse.bacc as bacc
import concourse.mybir as mybir
import concourse.tile as tile
from concourse import library_config
from concourse.bass_utils import run_bass_kernel_spmd

BF16 = ml_dtypes.bfloat16
F32 = np.float32

G, NPG, EPG, MAXN = 1024, 40, 80, 51
D_IN, D_OUT, D_EDGE = 64, 32, 16
NCORES = 8
GPC = G // NCORES          # 128 graphs / core
NPC = GPC * NPG            # 5120 nodes / core
EPC = GPC * EPG            # 10240 edges / core
NTYPES = 512
VOC = 9 * 128              # stacked atom-embedding rows

# ---- weight blob layout (bf16 elements) ----
ATAB_N = VOC * 128                 # 147456
WTAB_N = 64 * NTYPES * 32          # 1048576
WRO_COLS = 2560 + 256 + 32 + 32 + 8 + 1   # w1,w2,w3,rootp,w4,w5 = 2889
WRO_N = 128 * WRO_COLS             # 369792
BLOB_N = ATAB_N + WTAB_N + WRO_N   # 1565824 (divisible by 8)
SH_N = BLOB_N // NCORES            # 195728

# ---- index blob layout ([16, n/16] int16 columns) ----
ENC_W = 9 * NPC // 16              # 2880
RG_W = GPC * 128 // 16             # 1024


def _wrap16(idx):
    """int16 index array -> [16, n/16] (16-partition wrap, unreplicated)."""
    idx = np.asarray(idx, np.int16)
    n = idx.shape[0]
    assert n % 16 == 0
    w = np.empty((16, n // 16), np.int16)
    for p in range(16):
        w[p, :] = idx[p::16]
    return w


def _build_program(C, use_cc=True):
    """Emit the SPMD Tile program. C = per-type capacity (multiple of 64).

    use_cc=True: weight blob arrives sharded [SH_N] per core and is
    AllGathered on device. use_cc=False (CoreSim): full blob input.
    """
    dt = mybir.dt
    nc = bacc.Bacc("TRN2", target_bir_lowering=False, debug=False)

    NXS = NTYPES * C           # type-padded edge columns
    NXT = NXS + NPC            # + identity (x.T) columns
    assert C % 64 == 0
    CHUNKS = NXS // 128        # msg psum chunks of 128 rows
    XT_W = NXT // 16
    IDX_W = ENC_W + XT_W + RG_W

    # ---- DRAM I/O ----
    if use_cc:
        wshard = nc.dram_tensor("wshard", [SH_N], dt.bfloat16,
                                kind="ExternalInput")
        wcc = nc.dram_tensor("wcc", [SH_N], dt.bfloat16)
        wfull = nc.dram_tensor("wfull", [BLOB_N], dt.bfloat16)
    else:
        wfull = nc.dram_tensor("wshard", [BLOB_N], dt.bfloat16,
                               kind="ExternalInput")
    fb = nc.dram_tensor("fb", [128, 7], dt.float32, kind="ExternalInput")
    idx16 = nc.dram_tensor("idx16", [16, IDX_W], dt.int16,
                           kind="ExternalInput")
    dstc = nc.dram_tensor("dstc", [128, GPC], dt.int16, kind="ExternalInput")

    x_dram = nc.dram_tensor("x_scr", [NPC + 128, 128], dt.bfloat16)
    msg_dram = nc.dram_tensor("msg_scr", [NXS, 64], dt.float32)
    y = nc.dram_tensor("y", [1, GPC], dt.float32, kind="ExternalOutput")

    with tile.TileContext(nc) as tc:
        ch_reg = [None]

        def chunked_gather(dst3, srcT, idx, total, elem, transpose=False):
            CH = 512
            assert total % CH == 0
            if ch_reg[0] is None:
                ch_reg[0] = nc.gpsimd.to_reg(CH)
            for k in range(total // CH):
                isl = idx[:, k * (CH // 16):(k + 1) * (CH // 16)]
                if transpose:
                    osl = dst3[:, :, k * CH:(k + 1) * CH]
                else:
                    osl = dst3[:, k * (CH // 128):(k + 1) * (CH // 128), :]
                nc.gpsimd.dma_gather(osl, srcT, isl, CH, ch_reg[0], elem,
                                     transpose=transpose)

        nc.gpsimd.load_library(library_config.mlp)

        # ---- AllGather the weight blob (sharded upload) ----
        if use_cc:
            nc.gpsimd.dma_start(wcc[:], wshard[:])
            nc.gpsimd.collective_compute(
                "AllGather",
                mybir.AluOpType.bypass,
                replica_groups=[list(range(NCORES))],
                ins=[wcc[:]],
                outs=[wfull[:]],
            )
        atab_v = wfull[0:ATAB_N].rearrange("(v d) -> v d", v=VOC)
        wtab_v = wfull[ATAB_N:ATAB_N + WTAB_N].rearrange(
            "(p f) -> p f", p=64)
        wro_v = wfull[ATAB_N + WTAB_N:BLOB_N].rearrange(
            "(p f) -> p f", p=128)

        with tc.tile_pool(name="persist", bufs=1) as pp:
            # ---- persistent tiles ----
            wtab_t = pp.tile([64, NTYPES * 32], dt.bfloat16)
            nc.sync.dma_start(wtab_t[:], wtab_v)
            wtab = wtab_t[:].rearrange("p (t o) -> p t o", o=32)
            wro = pp.tile([128, WRO_COLS], dt.bfloat16)
            nc.sync.dma_start(wro[:], wro_v)
            fb_sb = pp.tile([128, 7], dt.float32)
            nc.sync.dma_start(fb_sb[:], fb[:])
            idx_sb = pp.tile([128, IDX_W], dt.int16)
            for k in range(8):
                nc.sync.dma_start(idx_sb[16 * k:16 * (k + 1), :], idx16[:])
            dstc_sb = pp.tile([128, GPC], dt.int16)
            nc.sync.dma_start(dstc_sb[:], dstc[:])
            F = pp.tile([32, NPC], dt.float32)

            rootp_sb = wro[:, 2848:2880]
            cbias_sb = fb_sb[0:32, 0:1]

            # ---- dhat one-hot from dst indices (exact 0/1 in fp32) ----
            dhat_sb = pp.tile([128, GPC, 40], dt.float32)
            for m in range(40):
                nc.vector.tensor_scalar(
                    dhat_sb[:, :, m], dstc_sb[:], float(m), None,
                    op0=mybir.AluOpType.is_equal)

            # ---- encoder: gather 9 embedding rows/node in 3 passes, sum ----
            with tc.tile_pool(name="enc", bufs=1) as ep:
                eidx = idx_sb[:, 0:ENC_W]
                S = NPC // 128  # 40 slots per feature column
                x_bf = ep.tile([128, S, 128], dt.bfloat16)
                NB = 3 * NPC
                for b in range(3):
                    epart = ep.tile([128, NB // 128, 128], dt.bfloat16,
                                    tag="epart")
                    chunked_gather(
                        epart[:], atab_v,
                        eidx[:, b * (NB // 16):(b + 1) * (NB // 16)],
                        NB, 128)
                    if b == 0:
                        nc.vector.tensor_tensor(
                            x_bf[:], epart[:, 0:S, :], epart[:, S:2 * S, :],
                            op=mybir.AluOpType.add)
                        nc.vector.tensor_tensor(
                            x_bf[:], x_bf[:], epart[:, 2 * S:3 * S, :],
                            op=mybir.AluOpType.add)
                    else:
                        for j in range(3):
                            nc.vector.tensor_tensor(
                                x_bf[:], x_bf[:], epart[:, j * S:(j + 1) * S, :],
                                op=mybir.AluOpType.add)
                # stage x rows (+ one zero block) to DRAM for the src-gather
                xv = x_dram.ap().rearrange("(s p) d -> p s d", p=128)
                nc.sync.dma_start(xv[:, 0:S, :], x_bf[:])
                zrow = ep.tile([128, 1, 128], dt.bfloat16)
                nc.vector.memset(zrow[:], 0.0)
                nc.sync.dma_start(xv[:, S:S + 1, :], zrow[:])

            # ---- transposed gather: XS_T (type-sorted) ++ x.T ----
            xtp_cm = tc.tile_pool(name="xtp", bufs=1)
            xp = xtp_cm.__enter__()
            xt = xp.tile([128, 1, NXT], dt.bfloat16)
            xidx = idx_sb[:, ENC_W:ENC_W + XT_W]
            chunked_gather(xt[:], x_dram[:], xidx, NXT, 128, transpose=True)
            xtv = xt[:, 0, :]

            # ---- F init: x @ root + conv_bias (transposed) ----
            psp_cm = tc.tile_pool(name="ps_mid", bufs=3, space="PSUM")
            psp = psp_cm.__enter__()
            for nch in range(NPC // 512):
                ps = psp.tile([32, 512], dt.float32, tag="xr")
                nc.tensor.matmul(ps[:], rootp_sb,
                                 xtv[:, NXS + nch * 512: NXS + (nch + 1) * 512],
                                 start=True, stop=True)
                nc.scalar.activation(F[:, nch * 512:(nch + 1) * 512], ps[:],
                                     mybir.ActivationFunctionType.Identity,
                                     bias=cbias_sb)

            # ---- messages: per-type matmuls, staged to DRAM ----
            with tc.tile_pool(name="msgp", bufs=6) as mp:
                msgv = msg_dram.ap().rearrange("(s p) d -> p s d", p=128)
                for ch in range(CHUNKS):
                    ps = psp.tile([128, 32], dt.float32, tag="msg")
                    for half in range(128 // 64):
                        col = ch * 128 + half * 64
                        nc.tensor.matmul(ps[half * 64:(half + 1) * 64, :],
                                         xtv[0:64, col:col + 64],
                                         wtab[:, col // C, :],
                                         start=True, stop=True)
                    st = mp.tile([128, 32], dt.float32, tag="stage")
                    nc.vector.tensor_copy(st[:], ps[:])
                    nc.sync.dma_start(msgv[:, ch, 0:32], st[:])
            psp_cm.__exit__(None, None, None)
            xtp_cm.__exit__(None, None, None)

            # ---- regather per graph (128 rows each) + scatter matmul ----
            with tc.tile_pool(name="scat", bufs=1) as sp:
                ridx = idx_sb[:, ENC_W + XT_W:IDX_W]
                gt = sp.tile([128, GPC, 64], dt.float32)
                chunked_gather(gt[:], msg_dram[:], ridx, GPC * 128, 64)
                psp_cm = tc.tile_pool(name="ps_sc", bufs=6, space="PSUM")
                psp = psp_cm.__enter__()
                for g in range(GPC):
                    ps = psp.tile([32, 40], dt.float32, tag="sc")
                    nc.tensor.matmul(ps[:], gt[:, g, 0:32], dhat_sb[:, g, :],
                                     start=True, stop=True)
                    nc.vector.tensor_tensor(F[:, g * 40:(g + 1) * 40],
                                            F[:, g * 40:(g + 1) * 40], ps[:],
                                            op=mybir.AluOpType.add)
                psp_cm.__exit__(None, None, None)

            # ---- fold F[32,5120] -> F2[128,1280] (bf16) ----
            with tc.tile_pool(name="ro", bufs=1) as rp:
                F2 = rp.tile([128, GPC * 10], dt.bfloat16)
                Fv = F[:].rearrange("p (g q j) -> p g q j", g=GPC, q=10)
                for j in range(4):
                    dst = F2[j * 32:(j + 1) * 32, :].rearrange(
                        "p (g q) -> p g q", g=GPC)
                    nc.vector.tensor_copy(dst, Fv[:, :, :, j])

                # ---- readout MLP (transposed, biases per-partition) ----
                w1_sb = wro[:, 0:2560].rearrange("p (q r) -> p q r", q=10)
                w2_sb = wro[:, 2560:2816].rearrange("p (h r) -> p h r", h=2)
                w3_sb = wro[:, 2816:2848]
                w4_sb = wro[0:32, 2880:2888]
                w5_sb = wro[0:8, 2888:2889]
                mb1_sb = fb_sb[:, 1:3]
                mb2_sb = fb_sb[:, 3:4]
                mb3_sb = fb_sb[0:32, 4:5]
                mb4_sb = fb_sb[0:8, 5:6]
                mb5_sb = fb_sb[0:1, 6:7]

                psp_cm = tc.tile_pool(name="ps_ro", bufs=2, space="PSUM")
                psp = psp_cm.__enter__()
                F2q = F2[:].rearrange("p (g q) -> p q g", q=10)
                a1 = rp.tile([128, 2, GPC], dt.bfloat16)
                for mh in range(2):
                    ps = psp.tile([128, GPC], dt.float32, tag="ro1")
                    for q in range(10):
                        nc.tensor.matmul(ps[:], w1_sb[:, q, mh * 128:(mh + 1) * 128],
                                         F2q[:, q, :], start=(q == 0), stop=(q == 9))
                    nc.scalar.activation(a1[:, mh, :], ps[:],
                                         mybir.ActivationFunctionType.Relu,
                                         bias=mb1_sb[:, mh:mh + 1])
                ps2 = psp.tile([128, GPC], dt.float32, tag="ro1")
                for h in range(2):
                    nc.tensor.matmul(ps2[:], w2_sb[:, h, :], a1[:, h, :],
                                     start=(h == 0), stop=(h == 1))
                a2 = rp.tile([128, GPC], dt.bfloat16)
                nc.scalar.activation(a2[:], ps2[:],
                                     mybir.ActivationFunctionType.Relu,
                                     bias=mb2_sb)
                ps3 = psp.tile([32, GPC], dt.float32, tag="ro2")
                nc.tensor.matmul(ps3[:], w3_sb, a2[:], start=True, stop=True)
                a3 = rp.tile([32, GPC], dt.bfloat16)
                nc.scalar.activation(a3[:], ps3[:],
                                     mybir.ActivationFunctionType.Relu,
                                     bias=mb3_sb)
                ps4 = psp.tile([8, GPC], dt.float32, tag="ro2")
                nc.tensor.matmul(ps4[:], w4_sb, a3[:], start=True, stop=True)
                a4 = rp.tile([8, GPC], dt.bfloat16)
                nc.scalar.activation(a4[:], ps4[:],
                                     mybir.ActivationFunctionType.Relu,
                                     bias=mb4_sb)
                ps5 = psp.tile([1, GPC], dt.float32, tag="ro2")
                nc.tensor.matmul(ps5[:], w5_sb, a4[:], start=True, stop=True)
                yv = rp.tile([1, GPC], dt.float32)
                nc.scalar.activation(yv[:], ps5[:],
                                     mybir.ActivationFunctionType.Identity,
                                     bias=mb5_sb)
                nc.sync.dma_start(y[:], yv[:])
                psp_cm.__exit__(None, None, None)

    nc.compile()
    return nc


def _host_prep(node_features, edge_features, edge_index, batch,
               atom_emb, bond_emb, gW1, gW2, gW3, root, conv_bias, mws, mbs,
               use_cc=True):
    """Build per-core input maps + pick type capacity C."""
    nf = np.asarray(node_features, np.int64)
    ef = np.asarray(edge_features, np.int64)
    src = np.asarray(edge_index, np.int64)[0]
    dst = np.asarray(edge_index, np.int64)[1]
    atom_emb = np.asarray(atom_emb, F32)
    bond_emb = np.asarray(bond_emb, F32)
    gW1 = np.asarray(gW1, F32); gW2 = np.asarray(gW2, F32); gW3 = np.asarray(gW3, F32)
    root = np.asarray(root, F32); conv_bias = np.asarray(conv_bias, F32)
    mws = [np.asarray(w, F32) for w in mws]
    mbs = [np.asarray(b, F32) for b in mbs]

    # ---- replicated weight blob ----
    atab = np.zeros((VOC, 128), BF16)
    atab[:, :64] = atom_emb.reshape(VOC, 64).astype(BF16)

    # host-computed 512-entry weight table (parameter-only transform)
    tt = np.arange(NTYPES)
    i0, i1, i2 = tt // 64, (tt // 8) % 8, tt % 8
    E = bond_emb[0, i0] + bond_emb[1, i1] + bond_emb[2, i2]   # [512,16]
    h = np.maximum(E @ gW1, 0.0)
    h = np.maximum(h @ gW2, 0.0)
    W = (h @ gW3).reshape(NTYPES, 64, 32)                     # [t,d,o]
    wtab_host = W.transpose(1, 0, 2).reshape(64, NTYPES * 32).astype(BF16)

    rootp = np.zeros((128, 32), F32)
    rootp[:64] = root
    # readout weights: w1 reordered [(j*32+oo), q, r] = mW1[(4q+j)*32+oo, r]
    w1r = mws[0][:1280].reshape(40, 32, 256).reshape(10, 4, 32, 256) \
        .transpose(1, 2, 0, 3).reshape(128, 2560)
    w2r = mws[1].reshape(2, 128, 128).transpose(1, 0, 2).reshape(128, 256)
    wro = np.zeros((128, WRO_COLS), F32)
    wro[:, 0:2560] = w1r
    wro[:, 2560:2816] = w2r
    wro[:, 2816:2848] = mws[2]            # w3 [128,32]
    wro[:, 2848:2880] = rootp
    wro[0:32, 2880:2888] = mws[3]         # w4 [32,8]
    wro[0:8, 2888:2889] = mws[4]          # w5 [8,1]
    blob = np.concatenate([atab.ravel(),
                           wtab_host.astype(BF16).ravel(),
                           wro.astype(BF16).ravel()])
    assert blob.shape[0] == BLOB_N

    fbp = np.zeros((128, 7), F32)
    fbp[0:32, 0] = conv_bias
    fbp[:, 1:3] = mbs[0].reshape(2, 128).T
    fbp[:, 3] = mbs[1]
    fbp[0:32, 4] = mbs[2]
    fbp[0:8, 5] = mbs[3]
    fbp[0, 6] = mbs[4][0]

    # ---- per-core data ----
    types = (ef[:, 0] * 64 + ef[:, 1] * 8 + ef[:, 2]).astype(np.int64)
    counts_all = np.zeros((NCORES, NTYPES), np.int64)
    for c in range(NCORES):
        counts_all[c] = np.bincount(types[c * EPC:(c + 1) * EPC], minlength=NTYPES)
    C = max(64, int(np.ceil(counts_all.max() / 64)) * 64)
    assert counts_all.min(axis=1).max() < C  # every core has a padded slot

    in_maps = []
    for c in range(NCORES):
        nsl = slice(c * NPC, (c + 1) * NPC)
        esl = slice(c * EPC, (c + 1) * EPC)
        nf_c = nf[nsl]
        t_c = types[esl]
        src_c = src[esl] - c * NPC
        dst_c = dst[esl] - c * NPC
        cnt = counts_all[c]

        # encoder gather indices, feature-column major
        eidx = (np.arange(9)[:, None] * 128 + nf_c.T).reshape(-1)   # [9*5120]

        # type-sort: edge e -> column t*C + rank
        order = np.argsort(t_c, kind="stable")
        rank = np.empty(EPC, np.int64)
        off = np.concatenate([[0], np.cumsum(cnt)[:-1]])
        rank[order] = np.arange(EPC) - off[t_c[order]]
        pos = t_c * C + rank                                        # [EPC]
        xs_idx = np.full(NTYPES * C, NPC, np.int64)                 # pad -> zero row
        xs_idx[pos] = src_c
        xt_i = np.concatenate([xs_idx, np.arange(NPC)])

        # regather: graph-order 128-row tiles (80 real + 48 pad)
        tmin = int(np.argmin(cnt))
        zslot = tmin * C + int(cnt[tmin])
        rg = np.full((GPC, 128), zslot, np.int64)
        rg[:, :80] = pos.reshape(GPC, 80)
        rg_i = rg.reshape(-1)

        idx16 = np.concatenate(
            [_wrap16(eidx), _wrap16(xt_i), _wrap16(rg_i)], axis=1)

        # compact dst: slot k (partition) x graph g -> dst node in 0..39
        dstc = np.full((128, GPC), -1, np.int16)
        dstc[:80, :] = (dst_c - np.repeat(np.arange(GPC), EPG) * NPG) \
            .reshape(GPC, EPG).T
        m = dict(fb=fbp, idx16=idx16, dstc=dstc)
        if use_cc:
            m["wshard"] = blob[c * SH_N:(c + 1) * SH_N]
        else:
            m["wshard"] = blob
        in_maps.append(m)
    return in_maps, C


def kernel(node_features, edge_features, edge_index, batch,
           atom_emb, bond_emb, gW1, gW2, gW3, root, conv_bias,
           mW1, mb1, mW2, mb2, mW3, mb3, mW4, mb4, mW5, mb5):
    in_maps, C = _host_prep(
        node_features, edge_features, edge_index, batch, atom_emb, bond_emb,
        gW1, gW2, gW3, root, conv_bias,
        [mW1, mW2, mW3, mW4, mW5], [mb1, mb2, mb3, mb4, mb5])
    nc = _build_program(C)
    res = run_bass_kernel_spmd(nc, in_maps, list(range(NCORES)))
    y = np.concatenate([r["y"].reshape(GPC) for r in res.results])
    return y.reshape(G, 1).astype(F32)
